# revision 6
# baseline (speedup 1.0000x reference)
"""BitLinear baseline (layernorm -> sign(W - mean(W)) GEMM -> *beta) on 8 TRN2 cores.

Sharding: data-parallel over tokens. Each core gets 1024 of the 8192 tokens
(x pre-transposed on host to [D_in, T_loc] so the contraction dim lands on
SBUF partitions), plus the full W^T (host layout transform only) and a
per-core column shard of W^T for the distributed mean/beta reduction
(two scalar AllReduces).

Device-side math (per core):
  mu  = mean(W)            via sharded partial sums + AllReduce
  beta= mean|W - mu|       via sharded partial abs-sums + AllReduce
  stats of x (sum, sum sq) via ones-vector matmuls on the tensor engine
  out[s,o] = a[s] * ( sum_i x[s,i]*sign(w[o,i]-mu)  -  mu_x[s]*colsum[o] )
  where a[s] = beta / sqrt(var[s]+eps); the rank-1 correction term is fused
  into the PSUM accumulation as an extra K=1 matmul; colsum[o] comes from a
  ones-row matmul over the sign tiles. Matmuls run in bf16 (sign values are
  exact in bf16), accumulation in fp32 PSUM.
"""

import numpy as np
from contextlib import ExitStack

from concourse import bass, bacc, tile, mybir
from concourse.bass_utils import run_bass_kernel_spmd

F32 = mybir.dt.float32
BF16 = mybir.dt.bfloat16
P = 128
LN_EPS = 1e-5

# Problem constants (hardcoded per contract).
B, S, D_IN, D_OUT = 4, 2048, 4096, 4096
N_CORES = 8
T_TOTAL = B * S
T_LOC = T_TOTAL // N_CORES


def build_program(n_cores, t_loc, d_in, d_out, oc_width=512):
    """Build + compile the per-core Bass program (SPMD; per-core data differs
    only through the input maps)."""
    n_it = d_in // P            # i tiles (contraction)
    n_st = t_loc // P           # s tiles (tokens)
    n_oc = d_out // oc_width    # output-feature chunks
    osh = d_out // n_cores      # W^T column shard width for stats
    inv_w = 1.0 / float(d_in * d_out)
    inv_d = 1.0 / float(d_in)
    groups = [list(range(n_cores))]

    nc = bacc.Bacc("TRN2", target_bir_lowering=False, debug=False,
                   num_devices=n_cores)
    xt = nc.dram_tensor("xt", [d_in, t_loc], F32, kind="ExternalInput").ap()
    wt = nc.dram_tensor("wt", [d_in, d_out], F32, kind="ExternalInput").ap()
    wsh = nc.dram_tensor("wsh", [d_in, osh], F32, kind="ExternalInput").ap()
    out = nc.dram_tensor("out", [t_loc, d_out], F32, kind="ExternalOutput").ap()

    with tile.TileContext(nc) as tc, ExitStack() as ctx:
        const = ctx.enter_context(tc.tile_pool(name="const", bufs=1))
        persist = ctx.enter_context(tc.tile_pool(name="persist", bufs=1))
        dram = ctx.enter_context(tc.tile_pool(name="dram", bufs=1, space="DRAM"))

        ones_col_f = const.tile([P, 1], F32, tag="ones_col_f")
        nc.vector.memset(ones_col_f[:], 1.0)
        ones_col_bf = const.tile([P, 1], BF16, tag="ones_col_bf")
        nc.vector.memset(ones_col_bf[:], 1.0)
        ones_row_f = const.tile([1, P], F32, tag="ones_row_f")
        nc.vector.memset(ones_row_f[:], 1.0)
        eps_c = const.tile([1, 1], F32, tag="eps_c")
        nc.vector.memset(eps_c[:], LN_EPS)

        neg_mu = persist.tile([P, 1], F32, tag="neg_mu")       # -mean(W), bcast
        beta_sb = persist.tile([1, 1], F32, tag="beta_sb")
        negmu_bf = persist.tile([1, t_loc], BF16, tag="negmu_bf")  # -mean_x[s]
        a_col = persist.tile([P, n_st], F32, tag="a_col")      # beta*rsig per tok

        # ---------------- Phase 1+2: W stats (mu, beta) over the shard ------
        with tc.tile_pool(name="wshard", bufs=1) as shard_pool, \
             tc.tile_pool(name="wstat", bufs=2) as wstat, \
             tc.tile_pool(name="ps12", bufs=2, space="PSUM") as ps12:
            shard_tiles = []
            sums = wstat.tile([P, n_it], F32, tag="sums")
            for j in range(n_it):
                t = shard_pool.tile([P, osh], F32, tag=f"sh{j}")
                nc.sync.dma_start(t[:], wsh[j * P:(j + 1) * P, :])
                shard_tiles.append(t)
                nc.vector.tensor_reduce(
                    sums[:, j:j + 1], t[:],
                    axis=mybir.AxisListType.X, op=mybir.AluOpType.add)
            tot_col = wstat.tile([P, 1], F32, tag="tot_col")
            nc.vector.tensor_reduce(
                tot_col[:], sums[:],
                axis=mybir.AxisListType.X, op=mybir.AluOpType.add)
            ps_tot = ps12.tile([1, 1], F32, tag="ps_tot")
            nc.tensor.matmul(ps_tot[:], ones_col_f[:], tot_col[:])
            sb_tot = wstat.tile([1, 1], F32, tag="sb_tot")
            nc.scalar.copy(sb_tot[:], ps_tot[:])

            ar1_in = dram.tile([1, 1], F32, tag="ar1_in")
            ar1_out = dram.tile([1, 1], F32, tag="ar1_out")
            nc.sync.dma_start(ar1_in[:], sb_tot[:])
            nc.gpsimd.collective_compute(
                "AllReduce", mybir.AluOpType.add, replica_groups=groups,
                ins=[ar1_in.opt()], outs=[ar1_out.opt()])
            tot_all = wstat.tile([1, 1], F32, tag="tot_all")
            nc.sync.dma_start(tot_all[:], ar1_out[:])

            ps_b = ps12.tile([P, 1], F32, tag="ps_b")
            nc.tensor.matmul(ps_b[:], ones_row_f[:], tot_all[:])
            nc.scalar.mul(neg_mu[:], ps_b[:], -inv_w)

            # beta partials: sum |w - mu| over the shard
            absums = wstat.tile([P, n_it], F32, tag="absums")
            for j in range(n_it):
                wd = wstat.tile([P, osh], F32, tag="wd")
                nc.vector.tensor_scalar_add(wd[:], shard_tiles[j][:], neg_mu[:])
                nc.vector.tensor_reduce(
                    absums[:, j:j + 1], wd[:],
                    axis=mybir.AxisListType.X, op=mybir.AluOpType.add,
                    apply_absolute_value=True)
            abst = wstat.tile([P, 1], F32, tag="abst")
            nc.vector.tensor_reduce(
                abst[:], absums[:],
                axis=mybir.AxisListType.X, op=mybir.AluOpType.add)
            ps_abs = ps12.tile([1, 1], F32, tag="ps_abs")
            nc.tensor.matmul(ps_abs[:], ones_col_f[:], abst[:])
            sb_abs = wstat.tile([1, 1], F32, tag="sb_abs")
            nc.scalar.copy(sb_abs[:], ps_abs[:])

            ar2_in = dram.tile([1, 1], F32, tag="ar2_in")
            ar2_out = dram.tile([1, 1], F32, tag="ar2_out")
            nc.sync.dma_start(ar2_in[:], sb_abs[:])
            nc.gpsimd.collective_compute(
                "AllReduce", mybir.AluOpType.add, replica_groups=groups,
                ins=[ar2_in.opt()], outs=[ar2_out.opt()])
            abs_all = wstat.tile([1, 1], F32, tag="abs_all")
            nc.sync.dma_start(abs_all[:], ar2_out[:])
            nc.scalar.mul(beta_sb[:], abs_all[:], inv_w)

        # ---------------- Phase 3: x load (bf16) + token stats --------------
        xbf_pool = ctx.enter_context(tc.tile_pool(name="xbf", bufs=1))
        xbf_tiles = []
        n_ch = (t_loc + 511) // 512
        with tc.tile_pool(name="xload", bufs=3) as xload, \
             tc.tile_pool(name="x2p", bufs=3) as x2p, \
             tc.tile_pool(name="statsb", bufs=2) as statsb, \
             tc.tile_pool(name="ps3", bufs=1, space="PSUM") as ps3:
            ps_s = ps3.tile([1, t_loc], F32, tag="ps_s")
            ps_s2 = ps3.tile([1, t_loc], F32, tag="ps_s2")
            for i in range(n_it):
                xf = xload.tile([P, t_loc], F32, tag="xf")
                nc.sync.dma_start(xf[:], xt[i * P:(i + 1) * P, :])
                xb = xbf_pool.tile([P, t_loc], BF16, tag=f"xb{i}")
                nc.vector.tensor_copy(xb[:], xf[:])
                x2 = x2p.tile([P, t_loc], BF16, tag="x2")
                nc.scalar.square(x2[:], xf[:])
                for c in range(n_ch):
                    sl = slice(c * 512, min((c + 1) * 512, t_loc))
                    nc.tensor.matmul(ps_s[:, sl], ones_col_bf[:], xb[:, sl],
                                     start=(i == 0), stop=(i == n_it - 1))
                    nc.tensor.matmul(ps_s2[:, sl], ones_col_bf[:], x2[:, sl],
                                     start=(i == 0), stop=(i == n_it - 1))
                xbf_tiles.append(xb)

            nc.scalar.mul(negmu_bf[:], ps_s[:], -inv_d)
            mu_row = statsb.tile([1, t_loc], F32, tag="mu_row")
            nc.scalar.mul(mu_row[:], ps_s[:], inv_d)
            ex2 = statsb.tile([1, t_loc], F32, tag="ex2")
            nc.scalar.mul(ex2[:], ps_s2[:], inv_d)
            musq = statsb.tile([1, t_loc], F32, tag="musq")
            nc.vector.tensor_mul(musq[:], mu_row[:], mu_row[:])
            var = statsb.tile([1, t_loc], F32, tag="var")
            nc.vector.tensor_sub(var[:], ex2[:], musq[:])
            sd = statsb.tile([1, t_loc], F32, tag="sd")
            nc.scalar.activation(sd[:], var[:],
                                 mybir.ActivationFunctionType.Sqrt,
                                 bias=eps_c[:])
            rsig = statsb.tile([1, t_loc], F32, tag="rsig")
            nc.vector.reciprocal(rsig[:], sd[:])
            a_row = statsb.tile([1, t_loc], F32, tag="a_row")
            nc.vector.tensor_scalar_mul(a_row[:], rsig[:], beta_sb[:])
            a_dram = dram.tile([1, t_loc], F32, tag="a_dram")
            nc.sync.dma_start(a_dram[:], a_row[:])
            nc.sync.dma_start(
                a_col[:], a_dram[0, :].rearrange("(t p) -> p t", p=P))

        # ---------------- Phase 4: main GEMM over o-chunks ------------------
        wload = ctx.enter_context(tc.tile_pool(name="wload", bufs=4))
        wbin_pool = ctx.enter_context(
            tc.tile_pool(name="wbin", bufs=min(n_it + 8, 2 * n_it)))
        cspool = ctx.enter_context(tc.tile_pool(name="cs", bufs=2))
        outsb = ctx.enter_context(tc.tile_pool(name="outsb", bufs=4))
        ps_main = ctx.enter_context(tc.tile_pool(name="ps4", bufs=4, space="PSUM"))
        ps_cs = ctx.enter_context(tc.tile_pool(name="ps4c", bufs=2, space="PSUM"))

        for oc in range(n_oc):
            o0 = oc * oc_width
            cs_ps = ps_cs.tile([1, oc_width], F32, tag="cs_ps")
            wbin_tiles = []
            for i in range(n_it):
                wf = wload.tile([P, oc_width], F32, tag="wf")
                nc.sync.dma_start(wf[:], wt[i * P:(i + 1) * P, o0:o0 + oc_width])
                wb = wbin_pool.tile([P, oc_width], BF16, tag="wb")
                nc.scalar.activation(wb[:], wf[:],
                                     mybir.ActivationFunctionType.Sign,
                                     bias=neg_mu[:])
                wbin_tiles.append(wb)
                nc.tensor.matmul(cs_ps[:], ones_col_bf[:], wb[:],
                                 start=(i == 0), stop=(i == n_it - 1))
            cs_row = cspool.tile([1, oc_width], BF16, tag="cs_row")
            nc.vector.tensor_copy(cs_row[:], cs_ps[:])

            for s in range(n_st):
                po = ps_main.tile([P, oc_width], F32, tag="po")
                for i in range(n_it):
                    nc.tensor.matmul(po[:], xbf_tiles[i][:, s * P:(s + 1) * P],
                                     wbin_tiles[i][:],
                                     start=(i == 0), stop=False)
                nc.tensor.matmul(po[:], negmu_bf[:, s * P:(s + 1) * P],
                                 cs_row[:], start=False, stop=True)
                ob = outsb.tile([P, oc_width], F32, tag="ob")
                nc.scalar.activation(ob[:], po[:],
                                     mybir.ActivationFunctionType.Copy,
                                     scale=a_col[:, s:s + 1])
                nc.sync.dma_start(out[s * P:(s + 1) * P, o0:o0 + oc_width], ob[:])

    nc.compile()
    return nc


_PROGRAM_CACHE = {}


def _get_program(key):
    if key not in _PROGRAM_CACHE:
        _PROGRAM_CACHE[key] = build_program(*key)
    return _PROGRAM_CACHE[key]


def kernel(x: np.ndarray, weight: np.ndarray) -> np.ndarray:
    assert x.shape == (B, S, D_IN) and weight.shape == (D_OUT, D_IN)
    nc = _get_program((N_CORES, T_LOC, D_IN, D_OUT))

    x2d = np.ascontiguousarray(x.reshape(T_TOTAL, D_IN), dtype=np.float32)
    wt_full = np.ascontiguousarray(weight.T, dtype=np.float32)  # [D_IN, D_OUT]
    osh = D_OUT // N_CORES

    in_maps = []
    for c in range(N_CORES):
        xt_c = np.ascontiguousarray(x2d[c * T_LOC:(c + 1) * T_LOC, :].T)
        wsh_c = np.ascontiguousarray(wt_full[:, c * osh:(c + 1) * osh])
        in_maps.append({"xt": xt_c, "wt": wt_full, "wsh": wsh_c})

    res = run_bass_kernel_spmd(nc, in_maps, list(range(N_CORES)), trace=False)
    out = np.concatenate([res.results[c]["out"] for c in range(N_CORES)], axis=0)
    return np.ascontiguousarray(out.reshape(B, S, D_OUT))


# revision 7
# speedup vs baseline: 1.1326x; 1.1326x over previous
"""BitLinear baseline (layernorm -> sign(W - mean(W)) GEMM -> *beta) on 8 TRN2 cores.

Sharding: data-parallel over tokens. Each core gets 1024 of the 8192 tokens
(x pre-transposed on host to [D_in, T_loc] bf16 so the contraction dim lands
on SBUF partitions), the full W^T fp32 (host layout transform only), and a
per-core column shard of W^T for the distributed W stats.

Device-side math (per core):
  One AllReduce of [sum(W), sum|W|, sum(sign(W))] over W shards gives
    mu   = sum/N
    beta = (sum|W| - mu*sum_sign)/N     (exact up to O(mu^2) ~ 1e-7 rel)
  x token stats (sum, sum sq) via ones-vector matmuls on the tensor engine.
  out[s,o] = a[s] * ( sum_i x[s,i]*sign(w[o,i]-mu)  -  mu_x[s]*colsum[o] )
  with a[s] = beta / sqrt(var[s]+eps). The rank-1 correction is fused into
  the PSUM accumulation as an extra K=1 matmul; colsum comes from a DVE
  log-tree over the sign tile + one ones-row matmul. Matmuls run in bf16
  (sign values exact in bf16), accumulation in fp32 PSUM.
"""

import numpy as np
from contextlib import ExitStack

from concourse import bass, bacc, tile, mybir
from concourse.bass_utils import run_bass_kernel_spmd

F32 = mybir.dt.float32
BF16 = mybir.dt.bfloat16
P = 128
LN_EPS = 1e-5

B, S, D_IN, D_OUT = 4, 2048, 4096, 4096
N_CORES = 8
T_TOTAL = B * S
T_LOC = T_TOTAL // N_CORES


def build_program(n_cores, t_loc, d_in, d_out, oc_width=512):
    n_it = d_in // P            # i tiles (contraction)
    n_st = t_loc // P           # s tiles (tokens)
    n_oc = d_out // oc_width    # output-feature chunks
    osh = d_out // n_cores      # W^T column shard width for stats
    inv_w = 1.0 / float(d_in * d_out)
    inv_d = 1.0 / float(d_in)
    groups = [list(range(n_cores))]
    AX = mybir.AxisListType.X
    ADD = mybir.AluOpType.add
    AF = mybir.ActivationFunctionType

    nc = bacc.Bacc("TRN2", target_bir_lowering=False, debug=False,
                   num_devices=n_cores)
    xt = nc.dram_tensor("xt", [d_in, t_loc], BF16, kind="ExternalInput").ap()
    wt = nc.dram_tensor("wt", [d_in, d_out], F32, kind="ExternalInput").ap()
    wsh = nc.dram_tensor("wsh", [d_in, osh], F32, kind="ExternalInput").ap()
    out = nc.dram_tensor("out", [t_loc, d_out], F32, kind="ExternalOutput").ap()

    with tile.TileContext(nc) as tc, ExitStack() as ctx:
        const = ctx.enter_context(tc.tile_pool(name="const", bufs=1))
        persist = ctx.enter_context(tc.tile_pool(name="persist", bufs=1))
        dram = ctx.enter_context(tc.tile_pool(name="dram", bufs=1, space="DRAM"))

        ones_col_f = const.tile([P, 1], F32, tag="ones_col_f")
        nc.vector.memset(ones_col_f[:], 1.0)
        ones_col_bf = const.tile([P, 1], BF16, tag="ones_col_bf")
        nc.vector.memset(ones_col_bf[:], 1.0)
        ones_row_f = const.tile([1, P], F32, tag="ones_row_f")
        nc.vector.memset(ones_row_f[:], 1.0)
        eps_c = const.tile([1, 1], F32, tag="eps_c")
        nc.vector.memset(eps_c[:], LN_EPS)
        zero_c = const.tile([P, 1], F32, tag="zero_c")
        nc.vector.memset(zero_c[:], 0.0)

        neg_mu = persist.tile([P, 1], F32, tag="neg_mu")
        beta_sb = persist.tile([1, 1], F32, tag="beta_sb")
        negmu_bf = persist.tile([1, t_loc], BF16, tag="negmu_bf")
        a_col = persist.tile([P, n_st], F32, tag="a_col")

        # ---------- Phase 1: sharded W stats, one AllReduce of [3] ----------
        with tc.tile_pool(name="wshard", bufs=1) as shard_pool, \
             tc.tile_pool(name="wstat", bufs=2) as wstat, \
             tc.tile_pool(name="ps12", bufs=1, space="PSUM") as ps12:
            sums = wstat.tile([P, n_it], F32, tag="sums")
            asums = wstat.tile([P, n_it], F32, tag="asums")
            ssums = wstat.tile([P, n_it], F32, tag="ssums")
            for j in range(n_it):
                t = shard_pool.tile([P, osh], F32, tag=f"sh{j}")
                nc.sync.dma_start(t[:], wsh[j * P:(j + 1) * P, :])
                nc.vector.tensor_reduce(sums[:, j:j + 1], t[:], axis=AX, op=ADD)
                nc.vector.tensor_reduce(asums[:, j:j + 1], t[:], axis=AX,
                                        op=ADD, apply_absolute_value=True)
                sg = wstat.tile([P, osh], BF16, tag="sg")
                nc.scalar.activation(sg[:], t[:], AF.Sign, bias=zero_c[:])
                nc.vector.tensor_reduce(ssums[:, j:j + 1], sg[:], axis=AX, op=ADD)
            s3 = wstat.tile([P, 3], F32, tag="s3")
            nc.vector.tensor_reduce(s3[:, 0:1], sums[:], axis=AX, op=ADD)
            nc.vector.tensor_reduce(s3[:, 1:2], asums[:], axis=AX, op=ADD)
            nc.vector.tensor_reduce(s3[:, 2:3], ssums[:], axis=AX, op=ADD)
            ps_tot = ps12.tile([1, 3], F32, tag="ps_tot")
            nc.tensor.matmul(ps_tot[:], ones_col_f[:], s3[:])
            sb_tot = wstat.tile([1, 3], F32, tag="sb_tot")
            nc.scalar.copy(sb_tot[:], ps_tot[:])

            ar_in = dram.tile([1, 3], F32, tag="ar_in")
            ar_out = dram.tile([1, 3], F32, tag="ar_out")
            nc.sync.dma_start(ar_in[:], sb_tot[:])
            nc.gpsimd.collective_compute(
                "AllReduce", ADD, replica_groups=groups,
                ins=[ar_in.opt()], outs=[ar_out.opt()])
            tot = wstat.tile([1, 3], F32, tag="tot")
            nc.sync.dma_start(tot[:], ar_out[:])

            # neg_mu = -sum/N broadcast across partitions
            ps_b = ps12.tile([P, 1], F32, tag="ps_b")
            nc.tensor.matmul(ps_b[:], ones_row_f[:], tot[:, 0:1])
            nc.scalar.mul(neg_mu[:], ps_b[:], -inv_w)
            # beta = (abs_sum - mu*sign_sum)/N
            mu_sb = wstat.tile([1, 1], F32, tag="mu_sb")
            nc.scalar.mul(mu_sb[:], tot[:, 0:1], inv_w)
            t1 = wstat.tile([1, 1], F32, tag="t1")
            nc.vector.tensor_mul(t1[:], mu_sb[:], tot[:, 2:3])
            t2 = wstat.tile([1, 1], F32, tag="t2")
            nc.vector.tensor_sub(t2[:], tot[:, 1:2], t1[:])
            nc.scalar.mul(beta_sb[:], t2[:], inv_w)

        # ---------- Phase 2: x load (bf16 direct) + token stats -------------
        xbf_pool = ctx.enter_context(tc.tile_pool(name="xbf", bufs=1))
        wload = ctx.enter_context(tc.tile_pool(name="wload", bufs=4))
        xbf_tiles = []
        wf0_tiles = []
        n_ch = (t_loc + 511) // 512
        with tc.tile_pool(name="x2p", bufs=3) as x2p, \
             tc.tile_pool(name="statsb", bufs=2) as statsb, \
             tc.tile_pool(name="ps3", bufs=1, space="PSUM") as ps3:
            ps_s = ps3.tile([1, t_loc], F32, tag="ps_s")
            ps_s2 = ps3.tile([1, t_loc], F32, tag="ps_s2")
            for i in range(n_it):
                xb = xbf_pool.tile([P, t_loc], BF16, tag=f"xb{i}")
                nc.sync.dma_start(xb[:], xt[i * P:(i + 1) * P, :])
                xbf_tiles.append(xb)
                # interleave chunk-0 W loads with x loads
                wf = wload.tile([P, oc_width], F32, tag="wf")
                nc.sync.dma_start(wf[:], wt[i * P:(i + 1) * P, 0:oc_width])
                wf0_tiles.append(wf)
                x2 = x2p.tile([P, t_loc], BF16, tag="x2")
                nc.scalar.square(x2[:], xb[:])
                for c in range(n_ch):
                    sl = slice(c * 512, min((c + 1) * 512, t_loc))
                    nc.tensor.matmul(ps_s[:, sl], ones_col_bf[:], xb[:, sl],
                                     start=(i == 0), stop=(i == n_it - 1))
                    nc.tensor.matmul(ps_s2[:, sl], ones_col_bf[:], x2[:, sl],
                                     start=(i == 0), stop=(i == n_it - 1))

            nc.scalar.mul(negmu_bf[:], ps_s[:], -inv_d)
            mu_row = statsb.tile([1, t_loc], F32, tag="mu_row")
            nc.scalar.mul(mu_row[:], ps_s[:], inv_d)
            ex2 = statsb.tile([1, t_loc], F32, tag="ex2")
            nc.scalar.mul(ex2[:], ps_s2[:], inv_d)
            musq = statsb.tile([1, t_loc], F32, tag="musq")
            nc.vector.tensor_mul(musq[:], mu_row[:], mu_row[:])
            var = statsb.tile([1, t_loc], F32, tag="var")
            nc.vector.tensor_sub(var[:], ex2[:], musq[:])
            sd = statsb.tile([1, t_loc], F32, tag="sd")
            nc.scalar.activation(sd[:], var[:], AF.Sqrt, bias=eps_c[:])
            rsig = statsb.tile([1, t_loc], F32, tag="rsig")
            nc.vector.reciprocal(rsig[:], sd[:])
            a_row = statsb.tile([1, t_loc], F32, tag="a_row")
            nc.vector.tensor_scalar_mul(a_row[:], rsig[:], beta_sb[:])
            a_dram = dram.tile([1, t_loc], F32, tag="a_dram")
            nc.sync.dma_start(a_dram[:], a_row[:])
            nc.sync.dma_start(
                a_col[:], a_dram[0, :].rearrange("(t p) -> p t", p=P))

        # ---------- Phase 3: main GEMM over o-chunks ------------------------
        wbin_pool = ctx.enter_context(tc.tile_pool(name="wbin", bufs=2))
        tree_pool = ctx.enter_context(tc.tile_pool(name="tree", bufs=1))
        cspool = ctx.enter_context(tc.tile_pool(name="cs", bufs=2))
        outsb = ctx.enter_context(tc.tile_pool(name="outsb", bufs=4))
        ps_main = ctx.enter_context(tc.tile_pool(name="ps4", bufs=4, space="PSUM"))
        ps_csp = ctx.enter_context(tc.tile_pool(name="ps4c", bufs=2, space="PSUM"))

        for oc in range(n_oc):
            o0 = oc * oc_width
            wb = wbin_pool.tile([P, n_it, oc_width], BF16, tag="wb")
            for i in range(n_it):
                if oc == 0:
                    wf = wf0_tiles[i]
                else:
                    wf = wload.tile([P, oc_width], F32, tag="wf")
                    nc.sync.dma_start(
                        wf[:], wt[i * P:(i + 1) * P, o0:o0 + oc_width])
                nc.scalar.activation(wb[:, i, :], wf[:], AF.Sign,
                                     bias=neg_mu[:])
            # colsum via log-tree of slab adds (values stay small ints: exact)
            ts = tree_pool.tile([P, n_it // 2, oc_width], BF16, tag="ts")
            half = n_it // 2
            nc.vector.tensor_add(ts[:, 0:half, :], wb[:, 0:half, :],
                                 wb[:, half:n_it, :])
            while half > 1:
                h2 = half // 2
                nc.vector.tensor_add(ts[:, 0:h2, :], ts[:, 0:h2, :],
                                     ts[:, h2:half, :])
                half = h2
            cs_ps = ps_csp.tile([1, oc_width], F32, tag="cs_ps")
            nc.tensor.matmul(cs_ps[:], ones_col_bf[:], ts[:, 0, :])
            cs_row = cspool.tile([1, oc_width], BF16, tag="cs_row")
            nc.vector.tensor_copy(cs_row[:], cs_ps[:])

            for s in range(n_st):
                po = ps_main.tile([P, oc_width], F32, tag="po")
                for i in range(n_it):
                    nc.tensor.matmul(po[:], xbf_tiles[i][:, s * P:(s + 1) * P],
                                     wb[:, i, :], start=(i == 0), stop=False)
                nc.tensor.matmul(po[:], negmu_bf[:, s * P:(s + 1) * P],
                                 cs_row[:], start=False, stop=True)
                ob = outsb.tile([P, oc_width], F32, tag="ob")
                nc.scalar.activation(ob[:], po[:], AF.Copy,
                                     scale=a_col[:, s:s + 1])
                nc.sync.dma_start(out[s * P:(s + 1) * P, o0:o0 + oc_width], ob[:])

    nc.compile()
    return nc


_PROGRAM_CACHE = {}


def _get_program(key):
    if key not in _PROGRAM_CACHE:
        _PROGRAM_CACHE[key] = build_program(*key)
    return _PROGRAM_CACHE[key]


def kernel(x: np.ndarray, weight: np.ndarray) -> np.ndarray:
    assert x.shape == (B, S, D_IN) and weight.shape == (D_OUT, D_IN)
    nc = _get_program((N_CORES, T_LOC, D_IN, D_OUT))
    bf16 = mybir.dt.np(BF16)

    x2d = np.ascontiguousarray(x.reshape(T_TOTAL, D_IN), dtype=np.float32)
    wt_full = np.ascontiguousarray(weight.T, dtype=np.float32)  # [D_IN, D_OUT]
    osh = D_OUT // N_CORES

    in_maps = []
    for c in range(N_CORES):
        xt_c = np.ascontiguousarray(
            x2d[c * T_LOC:(c + 1) * T_LOC, :].T).astype(bf16)
        wsh_c = np.ascontiguousarray(wt_full[:, c * osh:(c + 1) * osh])
        in_maps.append({"xt": xt_c, "wt": wt_full, "wsh": wsh_c})

    res = run_bass_kernel_spmd(nc, in_maps, list(range(N_CORES)), trace=False)
    out = np.concatenate([res.results[c]["out"] for c in range(N_CORES)], axis=0)
    return np.ascontiguousarray(out.reshape(B, S, D_OUT))


# revision 18
# speedup vs baseline: 1.2287x; 1.0848x over previous
"""BitLinear baseline (layernorm -> sign(W - mean(W)) GEMM -> *beta) on 8 TRN2 cores.

Sharding: data-parallel over tokens. Each core gets 1024 of the 8192 tokens
(x pre-transposed on host to [D_in, T_loc] bf16 so the contraction dim lands
on SBUF partitions) and the full W^T fp32, ROTATED per core by c*512 columns
so that the program's "chunk 0" is that core's private 1/8 column shard of W.
Chunk 0 thus doubles as the W-stats shard (one fp32 read of W per core total),
and the host un-rotates the per-core outputs.

Device-side math (per core):
  One AllReduce of [sum(W), sum|W|, count(W>=0)] over the chunk-0 shards:
    mu   = sum/N
    beta = (sum|W| - mu*(2*count - N))/N   (|w-mu| identity, error O(mu^2)~1e-7)
  x token stats (sum, sum sq) via ones-vector matmuls on the tensor engine,
  scheduled to fill the AllReduce latency window.
  out[s,o] = a[s]*raw[s,o] + b2[s]*colsum[o],  raw = x @ sign(W-mu)^T
  with a[s] = beta/sqrt(var[s]+eps), b2[s] = -a[s]*mean_x[s]. colsum comes
  from a DVE add-tree over the sign tile + one ones-row matmul; the rank-1
  correction is applied in the epilogue on the vector engine.
  Matmuls run in bf16 (sign values exact in bf16), accumulation in fp32 PSUM.
"""

import numpy as np
from contextlib import ExitStack

from concourse import bass, bacc, tile, mybir
from concourse.bass_utils import run_bass_kernel_spmd

F32 = mybir.dt.float32
BF16 = mybir.dt.bfloat16
P = 128
LN_EPS = 1e-5

B, S, D_IN, D_OUT = 4, 2048, 4096, 4096
N_CORES = 8
T_TOTAL = B * S
T_LOC = T_TOTAL // N_CORES


def build_program(n_cores, t_loc, d_in, d_out, oc_width=512):
    n_it = d_in // P            # i tiles (contraction)
    n_st = t_loc // P           # s tiles (tokens)
    n_oc = d_out // oc_width    # output-feature chunks; chunk 0 = stats shard
    assert n_oc == n_cores and d_out % (n_cores * oc_width) == 0 or True
    inv_w = 1.0 / float(d_in * d_out)
    inv_d = 1.0 / float(d_in)
    groups = [list(range(n_cores))]
    AX = mybir.AxisListType.X
    ADD = mybir.AluOpType.add
    AF = mybir.ActivationFunctionType

    nc = bacc.Bacc("TRN2", target_bir_lowering=False, debug=False,
                   num_devices=n_cores)
    xt = nc.dram_tensor("xt", [d_in, t_loc], BF16, kind="ExternalInput").ap()
    wt = nc.dram_tensor("wt", [d_in, d_out], F32, kind="ExternalInput").ap()
    out = nc.dram_tensor("out", [t_loc, d_out], F32, kind="ExternalOutput").ap()

    with tile.TileContext(nc) as tc, ExitStack() as ctx:
        const = ctx.enter_context(tc.tile_pool(name="const", bufs=1))
        persist = ctx.enter_context(tc.tile_pool(name="persist", bufs=1))
        dram = ctx.enter_context(tc.tile_pool(name="dram", bufs=1, space="DRAM"))

        ones_col_f = const.tile([P, 1], F32, tag="ones_col_f")
        nc.vector.memset(ones_col_f[:], 1.0)
        ones_col_bf = const.tile([P, 1], BF16, tag="ones_col_bf")
        nc.vector.memset(ones_col_bf[:], 1.0)
        ones_row_f = const.tile([1, P], F32, tag="ones_row_f")
        nc.vector.memset(ones_row_f[:], 1.0)
        eps_c = const.tile([1, 1], F32, tag="eps_c")
        nc.vector.memset(eps_c[:], LN_EPS)
        zero_c = const.tile([P, 1], F32, tag="zero_c")
        nc.vector.memset(zero_c[:], 0.0)

        neg_mu = persist.tile([P, 1], F32, tag="neg_mu")
        beta_sb = persist.tile([1, 1], F32, tag="beta_sb")
        a_col = persist.tile([P, n_st], F32, tag="a_col")
        b_col = persist.tile([P, n_st], F32, tag="b_col")

        xbf_pool = ctx.enter_context(tc.tile_pool(name="xbf", bufs=1))
        wload = ctx.enter_context(tc.tile_pool(name="wload", bufs=3))
        wbin_pool = ctx.enter_context(
            tc.tile_pool(name="wbin", bufs=n_it + min(16, n_it)))
        tree_pool = ctx.enter_context(tc.tile_pool(name="tree", bufs=2))
        cspool = ctx.enter_context(tc.tile_pool(name="cs", bufs=2))
        outsb = ctx.enter_context(tc.tile_pool(name="outsb", bufs=3))

        # ---- Phase 1: W stats from chunk-0 tiles (per-core rotated shard) --
        ps12_ctx = ExitStack()
        ps12 = ps12_ctx.enter_context(
            tc.tile_pool(name="ps12", bufs=1, space="PSUM"))
        with tc.tile_pool(name="wstat", bufs=1) as wstat, \
             tc.tile_pool(name="wscr", bufs=2) as wscr, \
             tc.tile_pool(name="wfs", bufs=4) as wfs_pool:
            sums = wstat.tile([P, n_it], F32, tag="sums")
            asums = wstat.tile([P, n_it], F32, tag="asums")
            gsums = wstat.tile([P, n_it], F32, tag="gsums")
            for i in range(n_it):
                wf = wfs_pool.tile([P, oc_width], F32, tag="wfs")
                nc.sync.dma_start(wf[:], wt[i * P:(i + 1) * P, 0:oc_width])
                # sum on DVE, |w| on ACT (accum_out), w>=0 count on GpSimd
                nc.vector.tensor_reduce(sums[:, i:i + 1], wf[:], axis=AX, op=ADD)
                sabs = wscr.tile([P, oc_width], BF16, tag="sabs")
                nc.scalar.activation(sabs[:], wf[:], AF.Abs, bias=zero_c[:],
                                     accum_out=asums[:, i:i + 1])
                sge = wscr.tile([P, oc_width], BF16, tag="sge")
                nc.vector.tensor_scalar(sge[:], wf[:], 0.0, 0.0,
                                        mybir.AluOpType.is_ge, ADD,
                                        accum_out=gsums[:, i:i + 1])
            s3 = wstat.tile([P, 3], F32, tag="s3")
            nc.vector.tensor_reduce(s3[:, 0:1], sums[:], axis=AX, op=ADD)
            nc.vector.tensor_reduce(s3[:, 1:2], asums[:], axis=AX, op=ADD)
            nc.vector.tensor_reduce(s3[:, 2:3], gsums[:], axis=AX, op=ADD)
            ps_tot = ps12.tile([1, 3], F32, tag="ps_tot")
            nc.tensor.matmul(ps_tot[:], ones_col_f[:], s3[:])
            sb_tot = wstat.tile([1, 3], F32, tag="sb_tot")
            nc.scalar.copy(sb_tot[:], ps_tot[:])
            ar_in = dram.tile([1, 3], F32, tag="ar_in")
            ar_out = dram.tile([1, 3], F32, tag="ar_out")
            nc.sync.dma_start(ar_in[:], sb_tot[:])
            nc.gpsimd.collective_compute(
                "AllReduce", ADD, replica_groups=groups,
                ins=[ar_in.opt()], outs=[ar_out.opt()])

        # ---- Phase 2: x load (bf16) + token stats on PE (fills AR window) --
        xbf_tiles = []
        n_ch = (t_loc + 511) // 512
        with tc.tile_pool(name="statsb", bufs=1) as statsb, \
             tc.tile_pool(name="x2p", bufs=2) as x2p, \
             tc.tile_pool(name="ps3", bufs=1, space="PSUM") as ps3:
            ps_s = ps3.tile([1, t_loc], F32, tag="ps_s")
            ps_s2 = ps3.tile([1, t_loc], F32, tag="ps_s2")
            for i in range(n_it):
                xb = xbf_pool.tile([P, t_loc], BF16, tag=f"xb{i}")
                nc.sync.dma_start(xb[:], xt[i * P:(i + 1) * P, :])
                xbf_tiles.append(xb)
                x2 = x2p.tile([P, t_loc], BF16, tag="x2")
                nc.scalar.square(x2[:], xb[:])
                for c in range(n_ch):
                    sl = slice(c * 512, min((c + 1) * 512, t_loc))
                    nc.tensor.matmul(ps_s[:, sl], ones_col_bf[:], xb[:, sl],
                                     start=(i == 0), stop=(i == n_it - 1))
                    nc.tensor.matmul(ps_s2[:, sl], ones_col_bf[:], x2[:, sl],
                                     start=(i == 0), stop=(i == n_it - 1))

            # ---- Post-AR scalars (PE-order: after the stats matmuls) ------
            tot = statsb.tile([1, 3], F32, tag="tot")
            nc.sync.dma_start(tot[:], ar_out[:])
            ps_b = ps12.tile([P, 1], F32, tag="ps_b")
            nc.tensor.matmul(ps_b[:], ones_row_f[:], tot[:, 0:1])
            nc.scalar.mul(neg_mu[:], ps_b[:], -inv_w)
            mu_sb = statsb.tile([1, 1], F32, tag="mu_sb")
            nc.scalar.mul(mu_sb[:], tot[:, 0:1], inv_w)
            sgn_t = statsb.tile([1, 1], F32, tag="sgn_t")
            nc.scalar.activation(sgn_t[:], tot[:, 2:3], AF.Copy,
                                 scale=2.0, bias=-float(d_in * d_out))
            t1 = statsb.tile([1, 1], F32, tag="t1")
            nc.vector.tensor_mul(t1[:], mu_sb[:], sgn_t[:])
            t2 = statsb.tile([1, 1], F32, tag="t2")
            nc.vector.tensor_sub(t2[:], tot[:, 1:2], t1[:])
            nc.scalar.mul(beta_sb[:], t2[:], inv_w)

            # ---- token-stat epilogue -> a_col, b_col ----------------------
            mu_row = statsb.tile([1, t_loc], F32, tag="mu_row")
            nc.scalar.mul(mu_row[:], ps_s[:], inv_d)
            ex2 = statsb.tile([1, t_loc], F32, tag="ex2")
            nc.scalar.mul(ex2[:], ps_s2[:], inv_d)
            musq = statsb.tile([1, t_loc], F32, tag="musq")
            nc.vector.tensor_mul(musq[:], mu_row[:], mu_row[:])
            nc.vector.tensor_sub(ex2[:], ex2[:], musq[:])          # var
            nc.scalar.activation(musq[:], ex2[:], AF.Sqrt, bias=eps_c[:])
            rsig = statsb.tile([1, t_loc], F32, tag="rsig")
            nc.vector.reciprocal(rsig[:], musq[:])
            a_row = statsb.tile([1, t_loc], F32, tag="a_row")
            nc.vector.tensor_scalar_mul(a_row[:], rsig[:], beta_sb[:])
            b_row = statsb.tile([1, t_loc], F32, tag="b_row")
            nc.vector.tensor_mul(b_row[:], mu_row[:], a_row[:])
            nc.scalar.mul(b_row[:], b_row[:], -1.0)
            ab_dram = dram.tile([2, t_loc], F32, tag="ab_dram")
            nc.sync.dma_start(ab_dram[0:1, :], a_row[:])
            nc.sync.dma_start(ab_dram[1:2, :], b_row[:])
            nc.sync.dma_start(
                a_col[:], ab_dram[0, :].rearrange("(t p) -> p t", p=P))
            nc.sync.dma_start(
                b_col[:], ab_dram[1, :].rearrange("(t p) -> p t", p=P))

        ps12_ctx.close()

        # ---- Phase 3: main GEMM over o-chunks ------------------------------
        ps_main = ctx.enter_context(tc.tile_pool(name="ps4", bufs=4, space="PSUM"))
        ps_csp = ctx.enter_context(tc.tile_pool(name="ps4c", bufs=2, space="PSUM"))

        for oc in range(n_oc):
            o0 = oc * oc_width
            wb = [wbin_pool.tile([P, oc_width], BF16, tag="wb", name="wb")
                  for _ in range(n_it)]
            for i in range(n_it):
                wf = wload.tile([P, oc_width], F32, tag="wf")
                nc.sync.dma_start(
                    wf[:], wt[i * P:(i + 1) * P, o0:o0 + oc_width])
                nc.scalar.activation(wb[i][:], wf[:], AF.Sign,
                                     bias=neg_mu[:])
            # colsum: grouped DVE adds (small ints, exact in bf16)
            ngrp = min(4, n_it)
            per = n_it // ngrp
            gacc = tree_pool.tile([P, ngrp, oc_width], BF16, tag="gacc")
            for g in range(ngrp):
                base = g * per
                if per == 1:
                    nc.vector.tensor_copy(gacc[:, g, :], wb[base][:])
                else:
                    nc.vector.tensor_add(gacc[:, g, :], wb[base][:],
                                         wb[base + 1][:])
                    for k in range(2, per):
                        nc.vector.tensor_add(gacc[:, g, :], gacc[:, g, :],
                                             wb[base + k][:])
            for g in range(1, ngrp):
                nc.vector.tensor_add(gacc[:, 0, :], gacc[:, 0, :],
                                     gacc[:, g, :])
            cs_ps = ps_csp.tile([1, oc_width], F32, tag="cs_ps")
            nc.tensor.matmul(cs_ps[:], ones_col_bf[:], gacc[:, 0, :])
            cs_row = cspool.tile([1, oc_width], F32, tag="cs_row")
            nc.vector.tensor_copy(cs_row[:], cs_ps[:])
            csb_ps = ps_csp.tile([P, oc_width], F32, tag="csb_ps")
            nc.tensor.matmul(csb_ps[:], ones_row_f[:], cs_row[:])

            for s in range(n_st):
                po = ps_main.tile([P, oc_width], F32, tag="po")
                for i in range(n_it):
                    nc.tensor.matmul(po[:], xbf_tiles[i][:, s * P:(s + 1) * P],
                                     wb[i][:],
                                     start=(i == 0), stop=(i == n_it - 1))
                tob = outsb.tile([P, oc_width], F32, tag="tob")
                nc.scalar.activation(tob[:], po[:], AF.Copy,
                                     scale=a_col[:, s:s + 1])
                ob = outsb.tile([P, oc_width], F32, tag="ob")
                nc.vector.scalar_tensor_tensor(
                    ob[:], csb_ps[:], b_col[:, s:s + 1], tob[:],
                    op0=mybir.AluOpType.mult, op1=ADD)
                nc.sync.dma_start(out[s * P:(s + 1) * P, o0:o0 + oc_width], ob[:])

    nc.compile()
    return nc


_PROGRAM_CACHE = {}


def _get_program(key):
    if key not in _PROGRAM_CACHE:
        _PROGRAM_CACHE[key] = build_program(*key)
    return _PROGRAM_CACHE[key]


def make_in_maps(x2d, weight, n_cores, t_loc, oc_width=512):
    """Host-side sharding: token shards of x^T in bf16; per-core W^T rotated
    by c*oc_width columns so program chunk 0 is core c's stats shard."""
    bf16 = mybir.dt.np(BF16)
    d_in = x2d.shape[1]
    wt_full = np.ascontiguousarray(weight.T, dtype=np.float32)
    in_maps = []
    for c in range(n_cores):
        xt_c = np.ascontiguousarray(
            x2d[c * t_loc:(c + 1) * t_loc, :].T).astype(bf16)
        wt_c = np.ascontiguousarray(np.roll(wt_full, -c * oc_width, axis=1))
        in_maps.append({"xt": xt_c, "wt": wt_c})
    return in_maps


def assemble_output(outs, n_cores, oc_width=512):
    """Un-rotate per-core outputs and concatenate token shards."""
    fixed = [np.roll(outs[c], c * oc_width, axis=1) for c in range(n_cores)]
    return np.concatenate(fixed, axis=0)


def kernel(x: np.ndarray, weight: np.ndarray) -> np.ndarray:
    assert x.shape == (B, S, D_IN) and weight.shape == (D_OUT, D_IN)
    nc = _get_program((N_CORES, T_LOC, D_IN, D_OUT))
    x2d = np.ascontiguousarray(x.reshape(T_TOTAL, D_IN), dtype=np.float32)
    in_maps = make_in_maps(x2d, weight, N_CORES, T_LOC)
    res = run_bass_kernel_spmd(nc, in_maps, list(range(N_CORES)), trace=False)
    out = assemble_output([res.results[c]["out"] for c in range(N_CORES)],
                          N_CORES)
    return np.ascontiguousarray(out.reshape(B, S, D_OUT))
